# revision 2
# baseline (speedup 1.0000x reference)
"""Trainium2 Bass kernel: causal multi-head attention (B=2, N=2048, C=2048, 16 heads).

Sharding: 16 heads split across 8 cores (2 heads/core, tensor parallel).
Each core computes q/k/v projections for its 2 heads, causal attention,
and its partial out-projection y_c = ctx_c @ wo_c.T (bf16 partials).
Host sums partials + bo.

v2 design ("flipped" attention, all-bf16 data):
  Phase 1: qT/kT [head_dim, tokens] via lhsT=w^T, rhs=x^T (bf16 weights/x,
  f32 PSUM). v^T likewise, then DMA-XBAR-transposed (16-bit transpose
  ucode) into V natural [tok, d] -- no PE transposes.
  Phase 2 per (b, q-chunk, head): S computed NATURAL [q, k] per 128-row
  q-tile (lhsT=qT tile, rhs=kT range, exact causal k range). The causal
  diagonal is masked additively on S in PSUM (DVE adds -340 so
  exp(scale*s - 30) ~ 0). exp on the Scalar engine emits row-sum
  denominators for free via accum_out. DVE: reduce partials, reciprocal,
  and scale E rows by 1/denom (per-partition scalar). Scaled E tiles are
  DMA-XBAR-transposed into E^T [k, q] layout, consumed by the AV matmul
  (causal-restricted q widths). ctx is already normalized; out-projection
  accumulates both heads in PSUM; y copied out bf16.
  This removes the baseline's 160 row-sum matmuls, 64 PE transposes, and
  all broadcast-reciprocal work from the PE/Vector critical path.
"""

import os
import numpy as np
import ml_dtypes

import concourse.bass as bass
import concourse.tile as tile
from concourse import bacc, mybir
from concourse import bass_utils

F32 = mybir.dt.float32
BF16 = mybir.dt.bfloat16
AF = mybir.ActivationFunctionType
ALU = mybir.AluOpType
AX = mybir.AxisListType

# problem dims (hardcoded per contract)
B = 2
N = 2048
C = 2048
HEADS = 16
HD = 128          # head dim
NCORES = 8
HPC = HEADS // NCORES  # heads per core = 2
E = HPC * HD      # per-core projection width = 256
BN = B * N        # 4096
P = 128
CT = C // P       # 16 contraction tiles
NCH = 512         # n-chunk width for projections
NCHUNKS = BN // NCH   # 8
QCW = 512         # q-chunk width in attention phase 2
QCHUNKS = N // QCW    # 4 per batch
KT_PER_B = N // P     # 16 k-tiles per batch
TOK_TILES = BN // P   # 32
QT_PER_B = N // P     # 16 q-tiles per batch
SCALE = float(HD) ** -0.5
MASKNEG = -340.0      # additive S mask: exp(scale*(s-340)) ~ e^-30 ~ 0

_CACHE = {}


def _build():
    nc = bacc.Bacc(
        "TRN2",
        target_bir_lowering=False,
        debug=False,
        enable_asserts=False,
        num_devices=NCORES,
    )

    xT = nc.dram_tensor("xT", [C, BN], BF16, kind="ExternalInput").ap()
    wqT = nc.dram_tensor("wqT", [C, E], BF16, kind="ExternalInput").ap()
    wkT = nc.dram_tensor("wkT", [C, E], BF16, kind="ExternalInput").ap()
    wvT = nc.dram_tensor("wvT", [C, E], BF16, kind="ExternalInput").ap()
    woT = nc.dram_tensor("woT", [E, C], BF16, kind="ExternalInput").ap()
    bqh = nc.dram_tensor("bqh", [HPC, P], F32, kind="ExternalInput").ap()
    bkh = nc.dram_tensor("bkh", [HPC, P], F32, kind="ExternalInput").ap()
    bvh = nc.dram_tensor("bvh", [HPC, P], F32, kind="ExternalInput").ap()
    maskneg_d = nc.dram_tensor("maskneg", [P, P], F32, kind="ExternalInput").ap()
    yp = nc.dram_tensor("yp", [BN, C], BF16, kind="ExternalOutput").ap()

    with tile.TileContext(nc) as tc:
        with tc.tile_pool(name="persist", bufs=1) as persist:
            # persistent per-core activations
            qT = persist.tile([P, HPC, B, N], BF16, tag="qT")
            kT = persist.tile([P, HPC, B, N], BF16, tag="kT")
            vN = persist.tile([P, TOK_TILES, E], BF16, tag="vN")
            maskneg_sb = persist.tile([P, P], F32, tag="maskneg")

            # ---------------- Phase 1: projections ----------------
            with tc.tile_pool(name="p1w", bufs=1) as wpool, \
                 tc.tile_pool(name="p1x", bufs=8) as xpool, \
                 tc.tile_pool(name="p1vt", bufs=2) as vtpool, \
                 tc.tile_pool(name="p1_ps", bufs=6, space="PSUM") as pps:
                wq_sb = wpool.tile([P, CT, E], BF16, tag="wq")
                wk_sb = wpool.tile([P, CT, E], BF16, tag="wk")
                wv_sb = wpool.tile([P, CT, E], BF16, tag="wv")
                bq_sb = wpool.tile([P, HPC], F32, tag="bq")
                bk_sb = wpool.tile([P, HPC], F32, tag="bk")
                bv_sb = wpool.tile([P, HPC], F32, tag="bv")

                # DMA priority: first weight pieces + chunk-0 x quarters
                # land first so the PE starts within a few us.
                nc.sync.dma_start(bq_sb[:], bqh.rearrange("h p -> p h"))
                nc.sync.dma_start(bk_sb[:], bkh.rearrange("h p -> p h"))
                nc.sync.dma_start(bv_sb[:], bvh.rearrange("h p -> p h"))
                wsrc = [(wq_sb, wqT), (wk_sb, wkT), (wv_sb, wvT)]
                xTr = xT.rearrange("(t p) n -> p t n", p=P)
                # interleave weight pieces and chunk-0 x pieces in
                # consumption order so the first matmuls start ASAP
                xh0 = []
                for piece in range(8):
                    for (dst, src) in wsrc:
                        srcr = src.rearrange("(t p) e -> p t e", p=P)
                        nc.sync.dma_start(
                            dst[:, piece * 2:(piece + 1) * 2, :],
                            srcr[:, piece * 2:(piece + 1) * 2, :],
                        )
                    xc = xpool.tile([P, 2, NCH], BF16, tag="xc")
                    nc.sync.dma_start(
                        xc[:], xTr[:, piece * 2:(piece + 1) * 2, 0:NCH])
                    xh0.append(xc)
                nc.sync.dma_start(maskneg_sb[:], maskneg_d)

                for ch in range(NCHUNKS):
                    b = ch // (N // NCH)
                    nn0 = (ch % (N // NCH)) * NCH  # within-batch token offset
                    n0 = ch * NCH                  # global token offset
                    if ch == 0:
                        xh = xh0
                    else:
                        xh = []
                        for piece in range(8):
                            xc = xpool.tile([P, 2, NCH], BF16, tag="xc")
                            nc.sync.dma_start(
                                xc[:], xTr[:, piece * 2:(piece + 1) * 2,
                                           n0:n0 + NCH])
                            xh.append(xc)

                    # 6 accumulators (q/k/v x 2 heads); c-tile outer loop so
                    # each x quarter is released after its 2 c-tiles.
                    accs = [pps.tile([P, NCH], F32, tag="pacc",
                                     name=f"pacc_{ch}_{i}")
                            for i in range(3 * HPC)]
                    for ct in range(CT):
                        xq = xh[ct // 2][:, ct % 2, :]
                        for wi, (wsb, _) in enumerate(wsrc):
                            for h in range(HPC):
                                nc.tensor.matmul(
                                    accs[wi * HPC + h][:],
                                    wsb[:, ct, h * HD:(h + 1) * HD],
                                    xq,
                                    start=(ct == 0),
                                    stop=(ct == CT - 1),
                                )

                    for h in range(HPC):
                        nc.scalar.activation(
                            qT[:, h, b, nn0:nn0 + NCH], accs[h][:],
                            AF.Identity, bias=bq_sb[:, h:h + 1], scale=1.0)
                        nc.scalar.activation(
                            kT[:, h, b, nn0:nn0 + NCH], accs[HPC + h][:],
                            AF.Identity, bias=bk_sb[:, h:h + 1], scale=1.0)
                        # v^T with bias -> bf16, then DMA-XBAR transpose to
                        # V natural [tok, d] (no PE involvement)
                        vt = vtpool.tile([P, NCH], BF16, tag="vt")
                        nc.scalar.activation(
                            vt[:], accs[2 * HPC + h][:],
                            AF.Identity, bias=bv_sb[:, h:h + 1], scale=1.0)
                        for ts in range(NCH // P):
                            nc.sync.dma_start(
                                vN[:, ch * (NCH // P) + ts,
                                   h * HD:(h + 1) * HD],
                                vt[:, ts * P:(ts + 1) * P],
                                transpose=True,
                            )

            # ---------------- Phase 2: attention + out-proj ----------------
            with tc.tile_pool(name="p2const", bufs=1) as cpool, \
                 tc.tile_pool(name="p2et", bufs=3) as etpool, \
                 tc.tile_pool(name="p2etT", bufs=2) as etTpool, \
                 tc.tile_pool(name="p2dp", bufs=6) as dppool, \
                 tc.tile_pool(name="p2ctx", bufs=4) as ctxpool, \
                 tc.tile_pool(name="p2y", bufs=3) as ysbpool, \
                 tc.tile_pool(name="p2s_ps", bufs=3, space="PSUM") as spool, \
                 tc.tile_pool(name="p2c_ps", bufs=2, space="PSUM") as cps, \
                 tc.tile_pool(name="p2y_ps", bufs=2, space="PSUM") as yps:
                wo_sb = cpool.tile([P, HPC, C], BF16, tag="wo")
                nc.sync.dma_start(wo_sb[:], woT.rearrange("(h p) f -> p h f", p=P))

                for b in range(B):
                    for qc in range(QCHUNKS):
                        nkt = 4 * qc + 4  # causal: k-tiles 0..4qc+3
                        # ---- stage A: S natural + exp(+denom) + scale +
                        #      DMA-transpose to E^T, per head ----
                        etTs = []
                        for h in range(HPC):
                            etT = etTpool.tile([P, KT_PER_B, QCW], BF16,
                                               tag="etT")
                            etTs.append(etT)
                            for qtl in range(4):
                                qt = 4 * qc + qtl      # q-tile in batch
                                ncols = (qt + 1) * P   # causal k extent
                                nchk = (ncols + QCW - 1) // QCW
                                et = etpool.tile([P, N], BF16, tag="et")
                                dparts = dppool.tile([P, 4], F32, tag="dp")
                                for j in range(nchk):
                                    c0 = j * QCW
                                    w = min(QCW, ncols - c0)
                                    sps = spool.tile([P, QCW], F32, tag="s")
                                    nc.tensor.matmul(
                                        sps[:, :w],
                                        qT[:, h, b, qt * P:(qt + 1) * P],
                                        kT[:, h, b, c0:c0 + w],
                                        start=True, stop=True,
                                    )
                                    if j == nchk - 1:
                                        # causal diagonal: additive -inf-ish
                                        # mask on S before exp
                                        d0 = ncols - P - c0
                                        nc.vector.scalar_tensor_tensor(
                                            out=sps[:, d0:d0 + P],
                                            in0=sps[:, d0:d0 + P],
                                            scalar=1.0,
                                            in1=maskneg_sb[:],
                                            op0=ALU.mult,
                                            op1=ALU.add,
                                        )
                                    nc.scalar.activation(
                                        et[:, c0:c0 + w], sps[:, :w],
                                        AF.Exp, scale=SCALE,
                                        accum_out=dparts[:, j:j + 1],
                                    )
                                dsum = dppool.tile([P, 1], F32, tag="ds")
                                nc.vector.tensor_reduce(
                                    out=dsum[:], in_=dparts[:, :nchk],
                                    axis=AX.X, op=ALU.add)
                                rcp = dppool.tile([P, 1], F32, tag="rc")
                                nc.vector.reciprocal_approx_fast(rcp[:], dsum[:])
                                nc.vector.tensor_scalar(
                                    out=et[:, :ncols], in0=et[:, :ncols],
                                    scalar1=rcp[:], scalar2=None,
                                    op0=ALU.mult)
                                for kt in range(qt + 1):
                                    nc.sync.dma_start(
                                        etT[:, kt, qtl * P:(qtl + 1) * P],
                                        et[:, kt * P:(kt + 1) * P],
                                        transpose=True,
                                    )

                        # ---- stage B: AV (ctx is pre-normalized) ----
                        ctx_tiles = []
                        for h in range(HPC):
                            ctxu_ps = cps.tile([P, QCW], F32, tag="ctxu")
                            for kt in range(nkt):
                                a = kt - 4 * qc
                                off = max(0, a) * P
                                nc.tensor.matmul(
                                    ctxu_ps[:, off:QCW],
                                    vN[:, b * KT_PER_B + kt,
                                       h * HD:(h + 1) * HD],
                                    etTs[h][:, kt, off:QCW],
                                    start=(kt == 0), stop=(kt == nkt - 1),
                                )
                            ctx = ctxpool.tile([P, QCW], BF16, tag="ctx")
                            nc.vector.tensor_copy(ctx[:], ctxu_ps[:])
                            ctx_tiles.append(ctx)

                        # ---- out-projection for this (b, qc) block ----
                        for nt in range(QCW // P):
                            y_sb = ysbpool.tile([P, C], BF16, tag="ysb")
                            for fc in range(C // 512):
                                y_ps = yps.tile([P, 512], F32, tag="yps")
                                for h in range(HPC):
                                    nc.tensor.matmul(
                                        y_ps[:],
                                        ctx_tiles[h][:, nt * P:(nt + 1) * P],
                                        wo_sb[:, h, fc * 512:(fc + 1) * 512],
                                        start=(h == 0), stop=(h == HPC - 1),
                                    )
                                nc.vector.tensor_copy(
                                    y_sb[:, fc * 512:(fc + 1) * 512],
                                    y_ps[:])
                            row0 = b * N + qc * QCW + nt * P
                            nc.sync.dma_start(yp[row0:row0 + P, :], y_sb[:])

    nc.compile()
    return nc


def _host_prep(x, wq, bq, wk, bk, wv, bv, wo):
    """Build the 8 per-core input maps (bf16 data, f32 biases/mask)."""
    bf16 = ml_dtypes.bfloat16
    x = np.asarray(x, dtype=np.float32)
    xT = np.ascontiguousarray(x.reshape(BN, C).T.astype(bf16))  # [C, BN]

    # additive causal mask for the diagonal [q=partition, k=free] 128-tile:
    # 0 where k <= q, MASKNEG where k > q (exp(scale*(s+MASKNEG)) ~ 0)
    kl = np.arange(P)[None, :]
    ql = np.arange(P)[:, None]
    maskneg = np.where(kl <= ql, 0.0, MASKNEG).astype(np.float32)

    in_maps = []
    for c in range(NCORES):
        e0 = c * E
        in_maps.append({
            "xT": xT,
            "wqT": np.ascontiguousarray(
                np.asarray(wq, np.float32)[e0:e0 + E, :].T.astype(bf16)),
            "wkT": np.ascontiguousarray(
                np.asarray(wk, np.float32)[e0:e0 + E, :].T.astype(bf16)),
            "wvT": np.ascontiguousarray(
                np.asarray(wv, np.float32)[e0:e0 + E, :].T.astype(bf16)),
            "woT": np.ascontiguousarray(
                np.asarray(wo, np.float32)[:, e0:e0 + E].T.astype(bf16)),
            "bqh": np.ascontiguousarray(
                np.asarray(bq, np.float32)[e0:e0 + E].reshape(HPC, P)),
            "bkh": np.ascontiguousarray(
                np.asarray(bk, np.float32)[e0:e0 + E].reshape(HPC, P)),
            "bvh": np.ascontiguousarray(
                np.asarray(bv, np.float32)[e0:e0 + E].reshape(HPC, P)),
            "maskneg": maskneg,
        })
    return in_maps


def _ensure_ntff_hook_module():
    """run_bass_kernel_spmd(trace=True) imports antenv.axon_hooks; provide a
    stub (hook=None -> tracing skipped gracefully) if the module is absent."""
    try:
        import antenv.axon_hooks  # noqa: F401
    except ImportError:
        import sys
        import types
        try:
            import antenv
        except ImportError:
            return
        mod = types.ModuleType("antenv.axon_hooks")
        state = {"hook": None}
        mod.set_axon_ntff_profile_hook = lambda h: state.__setitem__("hook", h)
        mod.get_axon_ntff_profile_hook = lambda: state["hook"]
        sys.modules["antenv.axon_hooks"] = mod
        antenv.axon_hooks = mod


def kernel(**inputs):
    _ensure_ntff_hook_module()
    if "nc" not in _CACHE:
        _CACHE["nc"] = _build()
    nc = _CACHE["nc"]

    in_maps = _host_prep(
        inputs["x"], inputs["wq"], inputs["bq"], inputs["wk"], inputs["bk"],
        inputs["wv"], inputs["bv"], inputs["wo"],
    )

    res = bass_utils.run_bass_kernel_spmd(
        nc, in_maps, core_ids=list(range(NCORES)),
        trace=bool(os.environ.get("BASS_TRACE")),
    )
    _CACHE["last_result"] = res

    y = np.zeros((BN, C), dtype=np.float32)
    for c in range(NCORES):
        y += res.results[c]["yp"].astype(np.float32)
    y += np.asarray(inputs["bo"], dtype=np.float32)
    return y.reshape(B, N, C)


# revision 6
# speedup vs baseline: 2.6160x; 2.6160x over previous
"""Trainium2 Bass kernel: causal multi-head attention (B=2, N=2048, C=2048, 16 heads).

Sharding: 16 heads split across 8 cores (2 heads/core, tensor parallel).
Each core computes q/k/v projections for its 2 heads, causal attention,
and its partial out-projection y_c = ctx_c @ wo_c.T (bf16 partials).
Host sums partials + bo.

v3: baseline S^T orientation (proven 94% PE occupancy) with:
  - all activations/weights in bf16 (same PE rate as f32r, 2x DVE rate,
    half the DMA/SBUF) -- f32 only in PSUM/biases/softmax sums
  - V^T -> V natural via batched DMA-XBAR 16-bit transposes (one 3D
    instruction per (chunk, head)) instead of 64 PE transposes + copies
  - softmax row sums: E tiles accumulated on DVE (even k-tiles) and the
    otherwise-idle Pool engine (odd k-tiles) into two f32 accumulators;
    2 ones-matmuls per (b,h,qchunk) replace the per-k-tile ones-matmul
    (160 -> 32 PE matmuls)
  - causal: S and AV matmuls width-restricted on diagonal tiles (exp of
    the stale S region is finite garbage that the full-width 0/1 mask
    multiply zeroes before E-accumulation/AV)
  - y partials written bf16 (half the PSUM->SBUF copy + DMA cost)
"""

import os
import numpy as np
import ml_dtypes

import concourse.bass as bass
import concourse.tile as tile
from concourse import bacc, mybir
from concourse import bass_utils

F32 = mybir.dt.float32
F32R = mybir.dt.float32r
BF16 = mybir.dt.bfloat16
AF = mybir.ActivationFunctionType

# problem dims (hardcoded per contract)
B = 2
N = 2048
C = 2048
HEADS = 16
HD = 128          # head dim
NCORES = 8
HPC = HEADS // NCORES  # heads per core = 2
E = HPC * HD      # per-core projection width = 256
BN = B * N        # 4096
P = 128
CT = C // P       # 16 contraction tiles
NCH = 512         # n-chunk width for projections
NCHUNKS = BN // NCH   # 8
QCW = 512         # q-chunk width in attention
QCHUNKS = N // QCW    # 4 per batch
KT_PER_B = N // P     # 16 k-tiles per batch
TOK_TILES = BN // P   # 32
SCALE = float(HD) ** -0.5

_CACHE = {}


def _build():
    nc = bacc.Bacc(
        "TRN2",
        target_bir_lowering=False,
        debug=False,
        enable_asserts=False,
        num_devices=NCORES,
    )

    xT = nc.dram_tensor("xT", [C, BN], BF16, kind="ExternalInput").ap()
    wqT = nc.dram_tensor("wqT", [C, E], BF16, kind="ExternalInput").ap()
    wkT = nc.dram_tensor("wkT", [C, E], BF16, kind="ExternalInput").ap()
    wvT = nc.dram_tensor("wvT", [C, E], BF16, kind="ExternalInput").ap()
    woT = nc.dram_tensor("woT", [E, C], BF16, kind="ExternalInput").ap()
    bqh = nc.dram_tensor("bqh", [HPC, P], F32, kind="ExternalInput").ap()
    bkh = nc.dram_tensor("bkh", [HPC, P], F32, kind="ExternalInput").ap()
    bvh = nc.dram_tensor("bvh", [HPC, P], F32, kind="ExternalInput").ap()
    masks = nc.dram_tensor("masks", [4, P, QCW], BF16, kind="ExternalInput").ap()
    ones_d = nc.dram_tensor("ones_d", [P, P], F32R, kind="ExternalInput").ap()
    yp = nc.dram_tensor("yp", [BN, C], BF16, kind="ExternalOutput").ap()

    with tile.TileContext(nc) as tc:
        with tc.tile_pool(name="persist", bufs=1) as persist:
            # persistent per-core activations
            qT = persist.tile([P, HPC, B, N], BF16, tag="qT")
            kT = persist.tile([P, HPC, B, N], BF16, tag="kT")
            vN = persist.tile([P, TOK_TILES, E], BF16, tag="vN")
            masks_sb = persist.tile([P, 4, QCW], BF16, tag="masks")
            ones_sb = persist.tile([P, P], F32R, tag="ones")

            # ---------------- Phase 1: projections ----------------
            with tc.tile_pool(name="p1w", bufs=1) as wpool, \
                 tc.tile_pool(name="p1x", bufs=8) as xpool, \
                 tc.tile_pool(name="p1vt", bufs=3) as vtpool, \
                 tc.tile_pool(name="p1_ps", bufs=6, space="PSUM") as pps:
                wq_sb = wpool.tile([P, CT, E], BF16, tag="wq")
                wk_sb = wpool.tile([P, CT, E], BF16, tag="wk")
                wv_sb = wpool.tile([P, CT, E], BF16, tag="wv")
                bq_sb = wpool.tile([P, HPC], F32, tag="bq")
                bk_sb = wpool.tile([P, HPC], F32, tag="bk")
                bv_sb = wpool.tile([P, HPC], F32, tag="bv")

                # DMA priority: first weight pieces + chunk-0 x quarters
                # land first so the PE starts within a few us.
                nc.sync.dma_start(bq_sb[:], bqh.rearrange("h p -> p h"))
                nc.sync.dma_start(bk_sb[:], bkh.rearrange("h p -> p h"))
                nc.sync.dma_start(bv_sb[:], bvh.rearrange("h p -> p h"))
                wsrc = [(wq_sb, wqT), (wk_sb, wkT), (wv_sb, wvT)]
                xTr = xT.rearrange("(t p) n -> p t n", p=P)
                # interleave weight pieces and chunk-0 x pieces in
                # consumption order so the first matmuls start ASAP
                xh0 = []
                for piece in range(8):
                    for (dst, src) in wsrc:
                        srcr = src.rearrange("(t p) e -> p t e", p=P)
                        nc.sync.dma_start(
                            dst[:, piece * 2:(piece + 1) * 2, :],
                            srcr[:, piece * 2:(piece + 1) * 2, :],
                        )
                    xc = xpool.tile([P, 2, NCH], BF16, tag="xc")
                    nc.sync.dma_start(
                        xc[:], xTr[:, piece * 2:(piece + 1) * 2, 0:NCH])
                    xh0.append(xc)
                nc.sync.dma_start(masks_sb[:], masks.rearrange("a p n -> p a n"))
                nc.sync.dma_start(ones_sb[:], ones_d)

                for ch in range(NCHUNKS):
                    b = ch // (N // NCH)
                    nn0 = (ch % (N // NCH)) * NCH  # within-batch token offset
                    n0 = ch * NCH                  # global token offset
                    if ch == 0:
                        xh = xh0
                    else:
                        xh = []
                        for piece in range(8):
                            xc = xpool.tile([P, 2, NCH], BF16, tag="xc")
                            nc.sync.dma_start(
                                xc[:], xTr[:, piece * 2:(piece + 1) * 2,
                                           n0:n0 + NCH])
                            xh.append(xc)

                    # 6 accumulators (q/k/v x 2 heads); c-tile outer loop so
                    # each x quarter is released after its 2 c-tiles.
                    accs = [pps.tile([P, NCH], F32, tag="pacc",
                                     name=f"pacc_{ch}_{i}")
                            for i in range(3 * HPC)]
                    for ct in range(CT):
                        xq = xh[ct // 2][:, ct % 2, :]
                        for wi, (wsb, _) in enumerate(wsrc):
                            for h in range(HPC):
                                nc.tensor.matmul(
                                    accs[wi * HPC + h][:],
                                    wsb[:, ct, h * HD:(h + 1) * HD],
                                    xq,
                                    start=(ct == 0),
                                    stop=(ct == CT - 1),
                                )

                    for h in range(HPC):
                        nc.scalar.activation(
                            qT[:, h, b, nn0:nn0 + NCH], accs[h][:],
                            AF.Identity, bias=bq_sb[:, h:h + 1], scale=1.0)
                        nc.scalar.activation(
                            kT[:, h, b, nn0:nn0 + NCH], accs[HPC + h][:],
                            AF.Identity, bias=bk_sb[:, h:h + 1], scale=1.0)
                        # v^T with bias -> bf16, then one batched DMA-XBAR
                        # transpose to V natural [tok, d] (4 tiles/instr)
                        vt = vtpool.tile([P, NCH], BF16, tag="vt")
                        nc.scalar.activation(
                            vt[:], accs[2 * HPC + h][:],
                            AF.Identity, bias=bv_sb[:, h:h + 1], scale=1.0)
                        nc.sync.dma_start(
                            vN[:, ch * (NCH // P):(ch + 1) * (NCH // P),
                               h * HD:(h + 1) * HD],
                            vt[:],
                            transpose=True,
                        )

            # ---------------- Phase 2: attention + out-proj ----------------
            with tc.tile_pool(name="p2const", bufs=1) as cpool, \
                 tc.tile_pool(name="p2e", bufs=6) as epool, \
                 tc.tile_pool(name="p2ea", bufs=4) as eapool, \
                 tc.tile_pool(name="p2ctx", bufs=6) as ctxpool, \
                 tc.tile_pool(name="p2sm", bufs=3) as smpool, \
                 tc.tile_pool(name="p2y", bufs=2) as ysbpool, \
                 tc.tile_pool(name="p2s_ps", bufs=2, space="PSUM") as spool, \
                 tc.tile_pool(name="p2c_ps", bufs=2, space="PSUM") as cps, \
                 tc.tile_pool(name="p2sb_ps", bufs=2, space="PSUM") as sbps, \
                 tc.tile_pool(name="p2y_ps", bufs=2, space="PSUM") as yps:
                wo_sb = cpool.tile([P, HPC, C], BF16, tag="wo")
                nc.sync.dma_start(wo_sb[:], woT.rearrange("(h p) f -> p h f", p=P))

                for b in range(B):
                    for qc in range(QCHUNKS):
                        nkt = 4 * qc + 4  # causal: k-tiles 0..4qc+3
                        ctx_tiles = []
                        for h in range(HPC):
                            ctxu_ps = cps.tile([P, QCW], F32, tag="ctxu")
                            sums_bc = sbps.tile([P, QCW], F32, tag="sumbc")
                            # E accumulated over k-tiles: even k-tiles on
                            # DVE, odd on the (idle) Pool engine
                            eacc = [
                                eapool.tile([P, QCW], F32R, tag="ea0",
                                            name=f"ea0_{b}_{qc}_{h}"),
                                eapool.tile([P, QCW], F32R, tag="ea1",
                                            name=f"ea1_{b}_{qc}_{h}"),
                            ]
                            for kt in range(nkt):
                                a = kt - 4 * qc
                                off = max(0, a) * P  # causal q offset
                                sps = spool.tile([P, QCW], F32, tag="s")
                                nc.tensor.matmul(
                                    sps[:],
                                    kT[:, h, b, kt * P:(kt + 1) * P],
                                    qT[:, h, b, qc * QCW:(qc + 1) * QCW],
                                    start=True, stop=True,
                                )
                                et = epool.tile([P, QCW], BF16, tag="e")
                                nc.scalar.activation(
                                    et[:], sps[:], AF.Exp, scale=SCALE
                                )
                                if a >= 0:  # diagonal tile: causal 0/1 mask
                                    nc.vector.tensor_mul(
                                        et[:], et[:], masks_sb[:, a, :]
                                    )
                                eng = nc.vector if kt % 2 == 0 else nc.gpsimd
                                if kt < 2:
                                    eng.tensor_copy(eacc[kt % 2][:], et[:])
                                else:
                                    eng.tensor_add(
                                        eacc[kt % 2][:], eacc[kt % 2][:], et[:])
                                nc.tensor.matmul(
                                    ctxu_ps[:, off:QCW],
                                    vN[:, b * KT_PER_B + kt, h * HD:(h + 1) * HD],
                                    et[:, off:QCW],
                                    start=(kt == 0), stop=(kt == nkt - 1),
                                )
                            # row sums broadcast across partitions via two
                            # all-ones matmuls (one per accumulator half)
                            nc.tensor.matmul(
                                sums_bc[:], ones_sb[:], eacc[0][:],
                                start=True, stop=False,
                            )
                            nc.tensor.matmul(
                                sums_bc[:], ones_sb[:], eacc[1][:],
                                start=False, stop=True,
                            )
                            recip_bc = smpool.tile([P, QCW], F32, tag="recipbc")
                            nc.vector.reciprocal_approx_fast(recip_bc[:], sums_bc[:])
                            ctx = ctxpool.tile([P, QCW], BF16, tag="ctx")
                            nc.vector.tensor_mul(ctx[:], ctxu_ps[:], recip_bc[:])
                            ctx_tiles.append(ctx)

                        # out-projection for this (b, qc) block of tokens
                        for nt in range(QCW // P):
                            y_sb = ysbpool.tile([P, C], BF16, tag="ysb")
                            for fc in range(C // 512):
                                y_ps = yps.tile([P, 512], F32, tag="yps")
                                for h in range(HPC):
                                    nc.tensor.matmul(
                                        y_ps[:],
                                        ctx_tiles[h][:, nt * P:(nt + 1) * P],
                                        wo_sb[:, h, fc * 512:(fc + 1) * 512],
                                        start=(h == 0), stop=(h == HPC - 1),
                                    )
                                if fc % 2 == 0:
                                    nc.vector.tensor_copy(
                                        y_sb[:, fc * 512:(fc + 1) * 512],
                                        y_ps[:])
                                else:
                                    nc.scalar.copy(
                                        y_sb[:, fc * 512:(fc + 1) * 512],
                                        y_ps[:])
                            row0 = b * N + qc * QCW + nt * P
                            nc.sync.dma_start(yp[row0:row0 + P, :], y_sb[:])

    nc.compile()
    return nc


def _host_prep(x, wq, bq, wk, bk, wv, bv, wo):
    """Build the 8 per-core input maps (bf16 data, f32 biases)."""
    bf16 = ml_dtypes.bfloat16
    x = np.asarray(x, dtype=np.float32)
    xT = np.ascontiguousarray(x.reshape(BN, C).T.astype(bf16))  # [C, BN]

    m = np.zeros((4, P, QCW), dtype=np.float32)
    kl = np.arange(P)[:, None]
    ql = np.arange(QCW)[None, :]
    for a in range(4):
        m[a] = (ql >= (P * a + kl)).astype(np.float32)
    m = m.astype(bf16)

    in_maps = []
    for c in range(NCORES):
        e0 = c * E
        in_maps.append({
            "xT": xT,
            "wqT": np.ascontiguousarray(
                np.asarray(wq, np.float32)[e0:e0 + E, :].T.astype(bf16)),
            "wkT": np.ascontiguousarray(
                np.asarray(wk, np.float32)[e0:e0 + E, :].T.astype(bf16)),
            "wvT": np.ascontiguousarray(
                np.asarray(wv, np.float32)[e0:e0 + E, :].T.astype(bf16)),
            "woT": np.ascontiguousarray(
                np.asarray(wo, np.float32)[:, e0:e0 + E].T.astype(bf16)),
            "bqh": np.ascontiguousarray(
                np.asarray(bq, np.float32)[e0:e0 + E].reshape(HPC, P)),
            "bkh": np.ascontiguousarray(
                np.asarray(bk, np.float32)[e0:e0 + E].reshape(HPC, P)),
            "bvh": np.ascontiguousarray(
                np.asarray(bv, np.float32)[e0:e0 + E].reshape(HPC, P)),
            "masks": m,
            "ones_d": np.ones((P, P), dtype=np.float32),
        })
    return in_maps


def _ensure_ntff_hook_module():
    """run_bass_kernel_spmd(trace=True) imports antenv.axon_hooks; provide a
    stub (hook=None -> tracing skipped gracefully) if the module is absent."""
    try:
        import antenv.axon_hooks  # noqa: F401
    except ImportError:
        import sys
        import types
        try:
            import antenv
        except ImportError:
            return
        mod = types.ModuleType("antenv.axon_hooks")
        state = {"hook": None}
        mod.set_axon_ntff_profile_hook = lambda h: state.__setitem__("hook", h)
        mod.get_axon_ntff_profile_hook = lambda: state["hook"]
        sys.modules["antenv.axon_hooks"] = mod
        antenv.axon_hooks = mod


def kernel(**inputs):
    _ensure_ntff_hook_module()
    if "nc" not in _CACHE:
        _CACHE["nc"] = _build()
    nc = _CACHE["nc"]

    in_maps = _host_prep(
        inputs["x"], inputs["wq"], inputs["bq"], inputs["wk"], inputs["bk"],
        inputs["wv"], inputs["bv"], inputs["wo"],
    )

    res = bass_utils.run_bass_kernel_spmd(
        nc, in_maps, core_ids=list(range(NCORES)),
        trace=bool(os.environ.get("BASS_TRACE")),
    )
    _CACHE["last_result"] = res

    y = np.zeros((BN, C), dtype=np.float32)
    for c in range(NCORES):
        y += res.results[c]["yp"].astype(np.float32)
    y += np.asarray(inputs["bo"], dtype=np.float32)
    return y.reshape(B, N, C)


# revision 15
# speedup vs baseline: 2.7518x; 1.0519x over previous
"""Trainium2 Bass kernel: causal multi-head attention (B=2, N=2048, C=2048, 16 heads).

Sharding: 16 heads split across 8 cores (2 heads/core, tensor parallel).
Each core computes q/k/v projections for its 2 heads, causal attention,
and its partial out-projection y_c = ctx_c @ wo_c.T (bf16 partials).
Host sums partials + bo.

v3: baseline S^T orientation (proven 94% PE occupancy) with:
  - all activations/weights in bf16 (same PE rate as f32r, 2x DVE rate,
    half the DMA/SBUF) -- f32 only in PSUM/biases/softmax sums
  - V^T -> V natural via batched DMA-XBAR 16-bit transposes (one 3D
    instruction per (chunk, head)) instead of 64 PE transposes + copies
  - softmax row sums: E tiles accumulated on DVE (even k-tiles) and the
    otherwise-idle Pool engine (odd k-tiles) into two f32 accumulators;
    2 ones-matmuls per (b,h,qchunk) replace the per-k-tile ones-matmul
    (160 -> 32 PE matmuls)
  - causal: S and AV matmuls width-restricted on diagonal tiles (exp of
    the stale S region is finite garbage that the full-width 0/1 mask
    multiply zeroes before E-accumulation/AV)
  - y partials written bf16 (half the PSUM->SBUF copy + DMA cost)
"""

import os
import numpy as np
import ml_dtypes

import concourse.bass as bass
import concourse.tile as tile
from concourse import bacc, mybir
from concourse import bass_utils

F32 = mybir.dt.float32
F32R = mybir.dt.float32r
BF16 = mybir.dt.bfloat16
AF = mybir.ActivationFunctionType

# problem dims (hardcoded per contract)
B = 2
N = 2048
C = 2048
HEADS = 16
HD = 128          # head dim
NCORES = 8
HPC = HEADS // NCORES  # heads per core = 2
E = HPC * HD      # per-core projection width = 256
BN = B * N        # 4096
P = 128
CT = C // P       # 16 contraction tiles
NCH = 512         # n-chunk width for projections
NCHUNKS = BN // NCH   # 8
QCW = 512         # q-chunk width in attention
QCHUNKS = N // QCW    # 4 per batch
KT_PER_B = N // P     # 16 k-tiles per batch
TOK_TILES = BN // P   # 32
SCALE = float(HD) ** -0.5

_CACHE = {}


def _build():
    nc = bacc.Bacc(
        "TRN2",
        target_bir_lowering=False,
        debug=False,
        enable_asserts=False,
        num_devices=NCORES,
    )

    xT = nc.dram_tensor("xT", [C, BN], BF16, kind="ExternalInput").ap()
    wqT = nc.dram_tensor("wqT", [C, E], BF16, kind="ExternalInput").ap()
    wkT = nc.dram_tensor("wkT", [C, E], BF16, kind="ExternalInput").ap()
    wvT = nc.dram_tensor("wvT", [C, E], BF16, kind="ExternalInput").ap()
    woT = nc.dram_tensor("woT", [E, C], BF16, kind="ExternalInput").ap()
    bqh = nc.dram_tensor("bqh", [HPC, P], F32, kind="ExternalInput").ap()
    bkh = nc.dram_tensor("bkh", [HPC, P], F32, kind="ExternalInput").ap()
    bvh = nc.dram_tensor("bvh", [HPC, P], F32, kind="ExternalInput").ap()
    masks = nc.dram_tensor("masks", [4, P, QCW], BF16, kind="ExternalInput").ap()
    ones_d = nc.dram_tensor("ones_d", [P, P], BF16, kind="ExternalInput").ap()
    yp = nc.dram_tensor("yp", [BN, C], BF16, kind="ExternalOutput").ap()

    with tile.TileContext(nc) as tc:
        with tc.tile_pool(name="persist", bufs=1) as persist:
            # persistent per-core activations
            qT = persist.tile([P, HPC, B, N], BF16, tag="qT")
            kT = persist.tile([P, HPC, B, N], BF16, tag="kT")
            vN = persist.tile([P, TOK_TILES, E], BF16, tag="vN")
            masks_sb = persist.tile([P, 4, QCW], BF16, tag="masks")
            ones_sb = persist.tile([P, P], BF16, tag="ones")

            # ---------------- Phase 1: projections ----------------
            with tc.tile_pool(name="p1w", bufs=1) as wpool, \
                 tc.tile_pool(name="p1x", bufs=8) as xpool, \
                 tc.tile_pool(name="p1vt", bufs=3) as vtpool, \
                 tc.tile_pool(name="p1_ps", bufs=6, space="PSUM") as pps:
                wq_sb = wpool.tile([P, CT, E], BF16, tag="wq")
                wk_sb = wpool.tile([P, CT, E], BF16, tag="wk")
                wv_sb = wpool.tile([P, CT, E], BF16, tag="wv")
                bq_sb = wpool.tile([P, HPC], F32, tag="bq")
                bk_sb = wpool.tile([P, HPC], F32, tag="bk")
                bv_sb = wpool.tile([P, HPC], F32, tag="bv")

                wsrc = [(wq_sb, wqT), (wk_sb, wkT), (wv_sb, wvT)]
                xTr = xT.rearrange("(t p) n -> p t n", p=P)
                # interleave weight pieces and chunk-0 x pieces in
                # consumption order so the first matmuls start ASAP; the
                # first piece is split at single-c-tile granularity and
                # low-priority constants (biases/masks/ones) go afterwards.
                xh0 = []
                wsrcr = [(dst, src.rearrange("(t p) e -> p t e", p=P))
                         for (dst, src) in wsrc]
                # piece 0, single-c-tile halves
                xc0 = xpool.tile([P, 2, NCH], BF16, tag="xc", name="xc0")
                xh0.append(xc0)
                for sub in range(2):
                    for (dst, srcr) in wsrcr:
                        nc.sync.dma_start(
                            dst[:, sub:sub + 1, :], srcr[:, sub:sub + 1, :])
                    nc.sync.dma_start(
                        xc0[:, sub:sub + 1, :], xTr[:, sub:sub + 1, 0:NCH])
                for piece in range(1, 8):
                    for (dst, srcr) in wsrcr:
                        nc.sync.dma_start(
                            dst[:, piece * 2:(piece + 1) * 2, :],
                            srcr[:, piece * 2:(piece + 1) * 2, :],
                        )
                    xc = xpool.tile([P, 2, NCH], BF16, tag="xc")
                    nc.sync.dma_start(
                        xc[:], xTr[:, piece * 2:(piece + 1) * 2, 0:NCH])
                    xh0.append(xc)
                    if piece == 1:
                        nc.sync.dma_start(bq_sb[:], bqh.rearrange("h p -> p h"))
                        nc.sync.dma_start(bk_sb[:], bkh.rearrange("h p -> p h"))
                        nc.sync.dma_start(bv_sb[:], bvh.rearrange("h p -> p h"))
                nc.sync.dma_start(masks_sb[:], masks.rearrange("a p n -> p a n"))
                nc.sync.dma_start(ones_sb[:], ones_d)

                for ch in range(NCHUNKS):
                    b = ch // (N // NCH)
                    nn0 = (ch % (N // NCH)) * NCH  # within-batch token offset
                    n0 = ch * NCH                  # global token offset
                    if ch == 0:
                        xh = xh0
                    else:
                        xh = []
                        for piece in range(8):
                            xc = xpool.tile([P, 2, NCH], BF16, tag="xc")
                            nc.sync.dma_start(
                                xc[:], xTr[:, piece * 2:(piece + 1) * 2,
                                           n0:n0 + NCH])
                            xh.append(xc)

                    # 6 accumulators (q/k/v x 2 heads); c-tile outer loop so
                    # each x quarter is released after its 2 c-tiles.
                    accs = [pps.tile([P, NCH], F32, tag="pacc",
                                     name=f"pacc_{ch}_{i}")
                            for i in range(3 * HPC)]
                    for ct in range(CT):
                        xq = xh[ct // 2][:, ct % 2, :]
                        for wi, (wsb, _) in enumerate(wsrc):
                            for h in range(HPC):
                                nc.tensor.matmul(
                                    accs[wi * HPC + h][:],
                                    wsb[:, ct, h * HD:(h + 1) * HD],
                                    xq,
                                    start=(ct == 0),
                                    stop=(ct == CT - 1),
                                )

                    for h in range(HPC):
                        nc.scalar.activation(
                            qT[:, h, b, nn0:nn0 + NCH], accs[h][:],
                            AF.Identity, bias=bq_sb[:, h:h + 1], scale=1.0)
                        nc.scalar.activation(
                            kT[:, h, b, nn0:nn0 + NCH], accs[HPC + h][:],
                            AF.Identity, bias=bk_sb[:, h:h + 1], scale=1.0)
                        # v^T with bias -> bf16, then one batched DMA-XBAR
                        # transpose to V natural [tok, d] (4 tiles/instr)
                        vt = vtpool.tile([P, NCH], BF16, tag="vt")
                        nc.scalar.activation(
                            vt[:], accs[2 * HPC + h][:],
                            AF.Identity, bias=bv_sb[:, h:h + 1], scale=1.0)
                        nc.sync.dma_start(
                            vN[:, ch * (NCH // P):(ch + 1) * (NCH // P),
                               h * HD:(h + 1) * HD],
                            vt[:],
                            transpose=True,
                        )

            # ---------------- Phase 2: attention + out-proj ----------------
            with tc.tile_pool(name="p2const", bufs=1) as cpool, \
                 tc.tile_pool(name="p2e", bufs=6) as epool, \
                 tc.tile_pool(name="p2ea", bufs=4) as eapool, \
                 tc.tile_pool(name="p2ctx", bufs=6) as ctxpool, \
                 tc.tile_pool(name="p2sm", bufs=3) as smpool, \
                 tc.tile_pool(name="p2y", bufs=2) as ysbpool, \
                 tc.tile_pool(name="p2s_ps", bufs=2, space="PSUM") as spool, \
                 tc.tile_pool(name="p2c_ps", bufs=2, space="PSUM") as cps, \
                 tc.tile_pool(name="p2sb_ps", bufs=2, space="PSUM") as sbps, \
                 tc.tile_pool(name="p2y_ps", bufs=2, space="PSUM") as yps:
                wo_sb = cpool.tile([P, HPC, C], BF16, tag="wo")
                nc.sync.dma_start(wo_sb[:], woT.rearrange("(h p) f -> p h f", p=P))

                for b in range(B):
                    for qc in range(QCHUNKS):
                        nkt = 4 * qc + 4  # causal: k-tiles 0..4qc+3
                        ctx_tiles = []
                        for h in range(HPC):
                            ctxu_ps = cps.tile([P, QCW], F32, tag="ctxu")
                            sums_bc = sbps.tile([P, QCW], F32, tag="sumbc")
                            # E accumulated over k-tiles into 4 bf16
                            # accumulators (chains <= 4 adds deep, so bf16
                            # rounding stays negligible after the exact f32
                            # partition-sum matmul). Even k-tiles on DVE,
                            # odd on the otherwise-idle Pool engine.
                            eacc = [
                                eapool.tile([P, QCW], BF16, tag=f"ea{i}",
                                            name=f"ea{i}_{b}_{qc}_{h}")
                                for i in range(4)
                            ]
                            for kt in range(nkt):
                                a = kt - 4 * qc
                                off = max(0, a) * P  # causal q offset
                                # columns [0, off) are fully masked: S, exp,
                                # E-acc, and AV all skip them (never read)
                                sps = spool.tile([P, QCW], F32, tag="s")
                                nc.tensor.matmul(
                                    sps[:, off:],
                                    kT[:, h, b, kt * P:(kt + 1) * P],
                                    qT[:, h, b, qc * QCW + off:(qc + 1) * QCW],
                                    start=True, stop=True,
                                )
                                et = epool.tile([P, QCW], BF16, tag="e")
                                nc.scalar.activation(
                                    et[:, off:], sps[:, off:],
                                    AF.Exp, scale=SCALE
                                )
                                if a >= 0:
                                    # diagonal: 0/1 triangle mask over the
                                    # 128-wide band [off, off+128)
                                    mw = (a + 1) * P
                                    nc.vector.tensor_mul(
                                        et[:, off:mw], et[:, off:mw],
                                        masks_sb[:, a, off:mw]
                                    )
                                eng = nc.vector if kt % 2 == 0 else nc.gpsimd
                                if kt < 4:
                                    if off > 0:
                                        eng.memzero(eacc[kt][:, :off])
                                    eng.tensor_copy(
                                        eacc[kt][:, off:], et[:, off:])
                                else:
                                    eng.tensor_add(
                                        eacc[kt % 4][:, off:],
                                        eacc[kt % 4][:, off:], et[:, off:])
                                nc.tensor.matmul(
                                    ctxu_ps[:, off:QCW],
                                    vN[:, b * KT_PER_B + kt, h * HD:(h + 1) * HD],
                                    et[:, off:QCW],
                                    start=(kt == 0), stop=(kt == nkt - 1),
                                )
                            # row sums broadcast across partitions via
                            # all-ones matmuls (one per accumulator)
                            for i in range(4):
                                nc.tensor.matmul(
                                    sums_bc[:], ones_sb[:], eacc[i][:],
                                    start=(i == 0), stop=(i == 3),
                                )
                            recip_bc = smpool.tile([P, QCW], F32, tag="recipbc")
                            nc.vector.reciprocal_approx_fast(recip_bc[:], sums_bc[:])
                            ctx = ctxpool.tile([P, QCW], BF16, tag="ctx")
                            nc.vector.tensor_mul(ctx[:], ctxu_ps[:], recip_bc[:])
                            ctx_tiles.append(ctx)

                        # out-projection for this (b, qc) block of tokens
                        for nt in range(QCW // P):
                            y_sb = ysbpool.tile([P, C], BF16, tag="ysb")
                            for fc in range(C // 512):
                                y_ps = yps.tile([P, 512], F32, tag="yps")
                                for h in range(HPC):
                                    nc.tensor.matmul(
                                        y_ps[:],
                                        ctx_tiles[h][:, nt * P:(nt + 1) * P],
                                        wo_sb[:, h, fc * 512:(fc + 1) * 512],
                                        start=(h == 0), stop=(h == HPC - 1),
                                    )
                                # PSUM->SBUF bf16 copies: alternate DVE and
                                # Scalar (Pool cannot access PSUM)
                                if fc % 2 == 0:
                                    nc.vector.tensor_copy(
                                        y_sb[:, fc * 512:(fc + 1) * 512],
                                        y_ps[:])
                                else:
                                    nc.scalar.copy(
                                        y_sb[:, fc * 512:(fc + 1) * 512],
                                        y_ps[:])
                            row0 = b * N + qc * QCW + nt * P
                            nc.sync.dma_start(yp[row0:row0 + P, :], y_sb[:])

    nc.compile()
    return nc


def _host_prep(x, wq, bq, wk, bk, wv, bv, wo):
    """Build the 8 per-core input maps (bf16 data, f32 biases)."""
    bf16 = ml_dtypes.bfloat16
    x = np.asarray(x, dtype=np.float32)
    xT = np.ascontiguousarray(x.reshape(BN, C).T.astype(bf16))  # [C, BN]

    m = np.zeros((4, P, QCW), dtype=np.float32)
    kl = np.arange(P)[:, None]
    ql = np.arange(QCW)[None, :]
    for a in range(4):
        m[a] = (ql >= (P * a + kl)).astype(np.float32)
    m = m.astype(bf16)

    in_maps = []
    for c in range(NCORES):
        e0 = c * E
        in_maps.append({
            "xT": xT,
            "wqT": np.ascontiguousarray(
                np.asarray(wq, np.float32)[e0:e0 + E, :].T.astype(bf16)),
            "wkT": np.ascontiguousarray(
                np.asarray(wk, np.float32)[e0:e0 + E, :].T.astype(bf16)),
            "wvT": np.ascontiguousarray(
                np.asarray(wv, np.float32)[e0:e0 + E, :].T.astype(bf16)),
            "woT": np.ascontiguousarray(
                np.asarray(wo, np.float32)[:, e0:e0 + E].T.astype(bf16)),
            "bqh": np.ascontiguousarray(
                np.asarray(bq, np.float32)[e0:e0 + E].reshape(HPC, P)),
            "bkh": np.ascontiguousarray(
                np.asarray(bk, np.float32)[e0:e0 + E].reshape(HPC, P)),
            "bvh": np.ascontiguousarray(
                np.asarray(bv, np.float32)[e0:e0 + E].reshape(HPC, P)),
            "masks": m,
            "ones_d": np.ones((P, P), dtype=bf16),
        })
    return in_maps


def _ensure_ntff_hook_module():
    """run_bass_kernel_spmd(trace=True) imports antenv.axon_hooks; provide a
    stub (hook=None -> tracing skipped gracefully) if the module is absent."""
    try:
        import antenv.axon_hooks  # noqa: F401
    except ImportError:
        import sys
        import types
        try:
            import antenv
        except ImportError:
            return
        mod = types.ModuleType("antenv.axon_hooks")
        state = {"hook": None}
        mod.set_axon_ntff_profile_hook = lambda h: state.__setitem__("hook", h)
        mod.get_axon_ntff_profile_hook = lambda: state["hook"]
        sys.modules["antenv.axon_hooks"] = mod
        antenv.axon_hooks = mod


def kernel(**inputs):
    _ensure_ntff_hook_module()
    if "nc" not in _CACHE:
        _CACHE["nc"] = _build()
    nc = _CACHE["nc"]

    in_maps = _host_prep(
        inputs["x"], inputs["wq"], inputs["bq"], inputs["wk"], inputs["bk"],
        inputs["wv"], inputs["bv"], inputs["wo"],
    )

    res = bass_utils.run_bass_kernel_spmd(
        nc, in_maps, core_ids=list(range(NCORES)),
        trace=bool(os.environ.get("BASS_TRACE")),
    )
    _CACHE["last_result"] = res

    y = np.zeros((BN, C), dtype=np.float32)
    for c in range(NCORES):
        y += res.results[c]["yp"].astype(np.float32)
    y += np.asarray(inputs["bo"], dtype=np.float32)
    return y.reshape(B, N, C)


# revision 18
# speedup vs baseline: 2.8196x; 1.0246x over previous
"""Trainium2 Bass kernel: causal multi-head attention (B=2, N=2048, C=2048, 16 heads).

Sharding: 16 heads split across 8 cores (2 heads/core, tensor parallel).
Each core computes q/k/v projections for its 2 heads, causal attention,
and its partial out-projection y_c = ctx_c @ wo_c.T (bf16 partials).
Host sums partials + bo.

v3: baseline S^T orientation (proven 94% PE occupancy) with:
  - all activations/weights in bf16 (same PE rate as f32r, 2x DVE rate,
    half the DMA/SBUF) -- f32 only in PSUM/biases/softmax sums
  - V^T -> V natural via batched DMA-XBAR 16-bit transposes (one 3D
    instruction per (chunk, head)) instead of 64 PE transposes + copies
  - softmax row sums: E tiles accumulated on DVE (even k-tiles) and the
    otherwise-idle Pool engine (odd k-tiles) into two f32 accumulators;
    2 ones-matmuls per (b,h,qchunk) replace the per-k-tile ones-matmul
    (160 -> 32 PE matmuls)
  - causal: S and AV matmuls width-restricted on diagonal tiles (exp of
    the stale S region is finite garbage that the full-width 0/1 mask
    multiply zeroes before E-accumulation/AV)
  - y partials written bf16 (half the PSUM->SBUF copy + DMA cost)
"""

import os
import numpy as np
import ml_dtypes

import concourse.bass as bass
import concourse.tile as tile
from concourse import bacc, mybir
from concourse import bass_utils

F32 = mybir.dt.float32
F32R = mybir.dt.float32r
BF16 = mybir.dt.bfloat16
AF = mybir.ActivationFunctionType

# problem dims (hardcoded per contract)
B = 2
N = 2048
C = 2048
HEADS = 16
HD = 128          # head dim
NCORES = 8
HPC = HEADS // NCORES  # heads per core = 2
E = HPC * HD      # per-core projection width = 256
BN = B * N        # 4096
P = 128
CT = C // P       # 16 contraction tiles
NCH = 512         # n-chunk width for projections
NCHUNKS = BN // NCH   # 8
QCW = 512         # q-chunk width in attention
QCHUNKS = N // QCW    # 4 per batch
KT_PER_B = N // P     # 16 k-tiles per batch
TOK_TILES = BN // P   # 32
SCALE = float(HD) ** -0.5

_CACHE = {}


def _build():
    nc = bacc.Bacc(
        "TRN2",
        target_bir_lowering=False,
        debug=False,
        enable_asserts=False,
        num_devices=NCORES,
    )

    xT = nc.dram_tensor("xT", [C, BN], BF16, kind="ExternalInput").ap()
    wqT = nc.dram_tensor("wqT", [C, E], BF16, kind="ExternalInput").ap()
    wkT = nc.dram_tensor("wkT", [C, E], BF16, kind="ExternalInput").ap()
    wvT = nc.dram_tensor("wvT", [C, E], BF16, kind="ExternalInput").ap()
    woT = nc.dram_tensor("woT", [E, C], BF16, kind="ExternalInput").ap()
    bqh = nc.dram_tensor("bqh", [HPC, P], F32, kind="ExternalInput").ap()
    bkh = nc.dram_tensor("bkh", [HPC, P], F32, kind="ExternalInput").ap()
    bvh = nc.dram_tensor("bvh", [HPC, P], F32, kind="ExternalInput").ap()
    masks = nc.dram_tensor("masks", [4, P, QCW], BF16, kind="ExternalInput").ap()
    ones_d = nc.dram_tensor("ones_d", [P, P], BF16, kind="ExternalInput").ap()
    yp = nc.dram_tensor("yp", [BN, C], BF16, kind="ExternalOutput").ap()

    with tile.TileContext(nc) as tc:
        with tc.tile_pool(name="persist", bufs=1) as persist:
            # persistent per-core activations
            qT = persist.tile([P, HPC, B, N], BF16, tag="qT")
            kT = persist.tile([P, HPC, B, N], BF16, tag="kT")
            vN = persist.tile([P, TOK_TILES, E], BF16, tag="vN")
            masks_sb = persist.tile([P, 4, QCW], BF16, tag="masks")
            ones_sb = persist.tile([P, P], BF16, tag="ones")

            # ---------------- Phase 1: projections ----------------
            with tc.tile_pool(name="p1w", bufs=1) as wpool, \
                 tc.tile_pool(name="p1x", bufs=8) as xpool, \
                 tc.tile_pool(name="p1vt", bufs=3) as vtpool, \
                 tc.tile_pool(name="p1_ps", bufs=6, space="PSUM") as pps:
                wq_sb = wpool.tile([P, CT, E], BF16, tag="wq")
                wk_sb = wpool.tile([P, CT, E], BF16, tag="wk")
                wv_sb = wpool.tile([P, CT, E], BF16, tag="wv")
                bq_sb = wpool.tile([P, HPC], F32, tag="bq")
                bk_sb = wpool.tile([P, HPC], F32, tag="bk")
                bv_sb = wpool.tile([P, HPC], F32, tag="bv")

                wsrc = [(wq_sb, wqT), (wk_sb, wkT), (wv_sb, wvT)]
                xTr = xT.rearrange("(t p) n -> p t n", p=P)
                # interleave weight pieces and chunk-0 x pieces in
                # consumption order so the first matmuls start ASAP; the
                # first piece is split at single-c-tile granularity and
                # low-priority constants (biases/masks/ones) go afterwards.
                xh0 = []
                wsrcr = [(dst, src.rearrange("(t p) e -> p t e", p=P))
                         for (dst, src) in wsrc]
                # piece 0, single-c-tile halves
                xc0 = xpool.tile([P, 2, NCH], BF16, tag="xc", name="xc0")
                xh0.append(xc0)
                for sub in range(2):
                    for (dst, srcr) in wsrcr:
                        nc.sync.dma_start(
                            dst[:, sub:sub + 1, :], srcr[:, sub:sub + 1, :])
                    nc.sync.dma_start(
                        xc0[:, sub:sub + 1, :], xTr[:, sub:sub + 1, 0:NCH])
                for piece in range(1, 8):
                    for (dst, srcr) in wsrcr:
                        nc.sync.dma_start(
                            dst[:, piece * 2:(piece + 1) * 2, :],
                            srcr[:, piece * 2:(piece + 1) * 2, :],
                        )
                    xc = xpool.tile([P, 2, NCH], BF16, tag="xc")
                    nc.sync.dma_start(
                        xc[:], xTr[:, piece * 2:(piece + 1) * 2, 0:NCH])
                    xh0.append(xc)
                    if piece == 1:
                        nc.sync.dma_start(bq_sb[:], bqh.rearrange("h p -> p h"))
                        nc.sync.dma_start(bk_sb[:], bkh.rearrange("h p -> p h"))
                        nc.sync.dma_start(bv_sb[:], bvh.rearrange("h p -> p h"))
                nc.sync.dma_start(masks_sb[:], masks.rearrange("a p n -> p a n"))
                nc.sync.dma_start(ones_sb[:], ones_d)

                for ch in range(NCHUNKS):
                    b = ch // (N // NCH)
                    nn0 = (ch % (N // NCH)) * NCH  # within-batch token offset
                    n0 = ch * NCH                  # global token offset
                    if ch == 0:
                        xh = xh0
                    else:
                        xh = []
                        for piece in range(8):
                            xc = xpool.tile([P, 2, NCH], BF16, tag="xc")
                            nc.sync.dma_start(
                                xc[:], xTr[:, piece * 2:(piece + 1) * 2,
                                           n0:n0 + NCH])
                            xh.append(xc)

                    # 6 accumulators (q/k/v x 2 heads); c-tile outer loop so
                    # each x quarter is released after its 2 c-tiles.
                    accs = [pps.tile([P, NCH], F32, tag="pacc",
                                     name=f"pacc_{ch}_{i}")
                            for i in range(3 * HPC)]
                    for ct in range(CT):
                        xq = xh[ct // 2][:, ct % 2, :]
                        for wi, (wsb, _) in enumerate(wsrc):
                            for h in range(HPC):
                                nc.tensor.matmul(
                                    accs[wi * HPC + h][:],
                                    wsb[:, ct, h * HD:(h + 1) * HD],
                                    xq,
                                    start=(ct == 0),
                                    stop=(ct == CT - 1),
                                )

                    for h in range(HPC):
                        nc.scalar.activation(
                            qT[:, h, b, nn0:nn0 + NCH], accs[h][:],
                            AF.Identity, bias=bq_sb[:, h:h + 1], scale=1.0)
                        nc.scalar.activation(
                            kT[:, h, b, nn0:nn0 + NCH], accs[HPC + h][:],
                            AF.Identity, bias=bk_sb[:, h:h + 1], scale=1.0)
                        # v^T with bias -> bf16, then one batched DMA-XBAR
                        # transpose to V natural [tok, d] (4 tiles/instr)
                        vt = vtpool.tile([P, NCH], BF16, tag="vt")
                        nc.scalar.activation(
                            vt[:], accs[2 * HPC + h][:],
                            AF.Identity, bias=bv_sb[:, h:h + 1], scale=1.0)
                        nc.sync.dma_start(
                            vN[:, ch * (NCH // P):(ch + 1) * (NCH // P),
                               h * HD:(h + 1) * HD],
                            vt[:],
                            transpose=True,
                        )

            # ---------------- Phase 2: attention + out-proj ----------------
            with tc.tile_pool(name="p2const", bufs=1) as cpool, \
                 tc.tile_pool(name="p2e", bufs=6) as epool, \
                 tc.tile_pool(name="p2ea", bufs=4) as eapool, \
                 tc.tile_pool(name="p2ctx", bufs=6) as ctxpool, \
                 tc.tile_pool(name="p2sm", bufs=3) as smpool, \
                 tc.tile_pool(name="p2y", bufs=2) as ysbpool, \
                 tc.tile_pool(name="p2s_ps", bufs=2, space="PSUM") as spool, \
                 tc.tile_pool(name="p2c_ps", bufs=2, space="PSUM") as cps, \
                 tc.tile_pool(name="p2sb_ps", bufs=2, space="PSUM") as sbps, \
                 tc.tile_pool(name="p2y_ps", bufs=2, space="PSUM") as yps:
                wo_sb = cpool.tile([P, HPC, C], BF16, tag="wo")
                nc.sync.dma_start(wo_sb[:], woT.rearrange("(h p) f -> p h f", p=P))

                def outproj(b, qc, ctx_tiles):
                    """out-projection for one (b, qc) block of 512 tokens"""
                    for nt in range(QCW // P):
                        y_sb = ysbpool.tile([P, C], BF16, tag="ysb")
                        for fc in range(C // 512):
                            y_ps = yps.tile([P, 512], F32, tag="yps")
                            for h in range(HPC):
                                nc.tensor.matmul(
                                    y_ps[:],
                                    ctx_tiles[h][:, nt * P:(nt + 1) * P],
                                    wo_sb[:, h, fc * 512:(fc + 1) * 512],
                                    start=(h == 0), stop=(h == HPC - 1),
                                )
                            # PSUM->SBUF bf16 copies: alternate DVE and
                            # Scalar (Pool cannot access PSUM)
                            if fc % 2 == 0:
                                nc.vector.tensor_copy(
                                    y_sb[:, fc * 512:(fc + 1) * 512],
                                    y_ps[:])
                            else:
                                nc.scalar.copy(
                                    y_sb[:, fc * 512:(fc + 1) * 512],
                                    y_ps[:])
                        row0 = b * N + qc * QCW + nt * P
                        nc.sync.dma_start(yp[row0:row0 + P, :], y_sb[:])

                pending = None  # deferred (b, qc, ctx_tiles) outproj
                for b in range(B):
                    for qc in range(QCHUNKS):
                        nkt = 4 * qc + 4  # causal: k-tiles 0..4qc+3
                        # E accumulated over k-tiles into 4 bf16 accumulators
                        # per head (chains <= 4 adds deep keep bf16 rounding
                        # negligible; the f32 partition-sum matmul is exact).
                        # Odd k-tiles (incl. the last) on DVE, even on the
                        # otherwise-idle Pool engine (memset+add: its plain
                        # copy is slow).
                        ctxu = [cps.tile([P, QCW], F32, tag="ctxu",
                                         name=f"ctxu{h}_{b}_{qc}")
                                for h in range(HPC)]
                        eacc = [[eapool.tile([P, QCW], BF16, tag=f"ea{h}{i}",
                                             name=f"ea{h}{i}_{b}_{qc}")
                                 for i in range(4)] for h in range(HPC)]
                        # both heads interleaved kt-major: each S->exp->mask
                        # ->eacc chain is covered by the other head's matmuls
                        for kt in range(nkt):
                            a = kt - 4 * qc
                            off = max(0, a) * P  # causal q offset
                            # columns [0, off) are fully masked: S, exp,
                            # E-acc, and AV all skip them (never read)
                            ets = []
                            for h in range(HPC):
                                sps = spool.tile([P, QCW], F32, tag="s")
                                nc.tensor.matmul(
                                    sps[:, off:],
                                    kT[:, h, b, kt * P:(kt + 1) * P],
                                    qT[:, h, b, qc * QCW + off:(qc + 1) * QCW],
                                    start=True, stop=True,
                                )
                                et = epool.tile([P, QCW], BF16, tag="e")
                                nc.scalar.activation(
                                    et[:, off:], sps[:, off:],
                                    AF.Exp, scale=SCALE
                                )
                                if a >= 0:
                                    # diagonal: 0/1 triangle mask over the
                                    # 128-wide band [off, off+128)
                                    mw = (a + 1) * P
                                    nc.vector.tensor_mul(
                                        et[:, off:mw], et[:, off:mw],
                                        masks_sb[:, a, off:mw]
                                    )
                                ea = eacc[h][kt % 4]
                                if kt % 2 == 1:
                                    if kt < 4:
                                        if off > 0:
                                            nc.vector.memzero(ea[:, :off])
                                        nc.vector.tensor_copy(
                                            ea[:, off:], et[:, off:])
                                    else:
                                        nc.vector.tensor_add(
                                            ea[:, off:], ea[:, off:],
                                            et[:, off:])
                                else:
                                    if kt < 4:
                                        nc.gpsimd.memzero(ea[:])
                                        nc.gpsimd.tensor_add(
                                            ea[:, off:], ea[:, off:],
                                            et[:, off:])
                                    else:
                                        nc.gpsimd.tensor_add(
                                            ea[:, off:], ea[:, off:],
                                            et[:, off:])
                                ets.append(et)
                            for h in range(HPC):
                                nc.tensor.matmul(
                                    ctxu[h][:, off:QCW],
                                    vN[:, b * KT_PER_B + kt,
                                       h * HD:(h + 1) * HD],
                                    ets[h][:, off:QCW],
                                    start=(kt == 0), stop=(kt == nkt - 1),
                                )

                        # software pipeline: the PREVIOUS chunk's
                        # out-projection goes here, between this chunk's
                        # attention stream and its softmax reduction -- the
                        # ~8us of out-proj matmuls cover the E-accumulator
                        # completion latency so the ones-matmuls don't stall
                        if pending is not None:
                            outproj(*pending)

                        ctx_tiles = []
                        for h in range(HPC):
                            sums_bc = sbps.tile([P, QCW], F32, tag="sumbc")
                            for i in range(4):
                                nc.tensor.matmul(
                                    sums_bc[:], ones_sb[:], eacc[h][i][:],
                                    start=(i == 0), stop=(i == 3),
                                )
                            recip_bc = smpool.tile([P, QCW], F32, tag="recipbc")
                            nc.vector.reciprocal_approx_fast(
                                recip_bc[:], sums_bc[:])
                            ctx = ctxpool.tile([P, QCW], BF16, tag="ctx")
                            nc.vector.tensor_mul(ctx[:], ctxu[h][:], recip_bc[:])
                            ctx_tiles.append(ctx)

                        pending = (b, qc, ctx_tiles)
                outproj(*pending)

    nc.compile()
    return nc


def _host_prep(x, wq, bq, wk, bk, wv, bv, wo):
    """Build the 8 per-core input maps (bf16 data, f32 biases)."""
    bf16 = ml_dtypes.bfloat16
    x = np.asarray(x, dtype=np.float32)
    xT = np.ascontiguousarray(x.reshape(BN, C).T.astype(bf16))  # [C, BN]

    m = np.zeros((4, P, QCW), dtype=np.float32)
    kl = np.arange(P)[:, None]
    ql = np.arange(QCW)[None, :]
    for a in range(4):
        m[a] = (ql >= (P * a + kl)).astype(np.float32)
    m = m.astype(bf16)

    in_maps = []
    for c in range(NCORES):
        e0 = c * E
        in_maps.append({
            "xT": xT,
            "wqT": np.ascontiguousarray(
                np.asarray(wq, np.float32)[e0:e0 + E, :].T.astype(bf16)),
            "wkT": np.ascontiguousarray(
                np.asarray(wk, np.float32)[e0:e0 + E, :].T.astype(bf16)),
            "wvT": np.ascontiguousarray(
                np.asarray(wv, np.float32)[e0:e0 + E, :].T.astype(bf16)),
            "woT": np.ascontiguousarray(
                np.asarray(wo, np.float32)[:, e0:e0 + E].T.astype(bf16)),
            "bqh": np.ascontiguousarray(
                np.asarray(bq, np.float32)[e0:e0 + E].reshape(HPC, P)),
            "bkh": np.ascontiguousarray(
                np.asarray(bk, np.float32)[e0:e0 + E].reshape(HPC, P)),
            "bvh": np.ascontiguousarray(
                np.asarray(bv, np.float32)[e0:e0 + E].reshape(HPC, P)),
            "masks": m,
            "ones_d": np.ones((P, P), dtype=bf16),
        })
    return in_maps


def _ensure_ntff_hook_module():
    """run_bass_kernel_spmd(trace=True) imports antenv.axon_hooks; provide a
    stub (hook=None -> tracing skipped gracefully) if the module is absent."""
    try:
        import antenv.axon_hooks  # noqa: F401
    except ImportError:
        import sys
        import types
        try:
            import antenv
        except ImportError:
            return
        mod = types.ModuleType("antenv.axon_hooks")
        state = {"hook": None}
        mod.set_axon_ntff_profile_hook = lambda h: state.__setitem__("hook", h)
        mod.get_axon_ntff_profile_hook = lambda: state["hook"]
        sys.modules["antenv.axon_hooks"] = mod
        antenv.axon_hooks = mod


def kernel(**inputs):
    _ensure_ntff_hook_module()
    if "nc" not in _CACHE:
        _CACHE["nc"] = _build()
    nc = _CACHE["nc"]

    in_maps = _host_prep(
        inputs["x"], inputs["wq"], inputs["bq"], inputs["wk"], inputs["bk"],
        inputs["wv"], inputs["bv"], inputs["wo"],
    )

    res = bass_utils.run_bass_kernel_spmd(
        nc, in_maps, core_ids=list(range(NCORES)),
        trace=bool(os.environ.get("BASS_TRACE")),
    )
    _CACHE["last_result"] = res

    y = np.zeros((BN, C), dtype=np.float32)
    for c in range(NCORES):
        y += res.results[c]["yp"].astype(np.float32)
    y += np.asarray(inputs["bo"], dtype=np.float32)
    return y.reshape(B, N, C)
